# revision 48
# baseline (speedup 1.0000x reference)
"""Multi-head causal attention with RoPE on 8 Trainium2 NeuronCores.

Sharding: batch x head-group. Core c owns batch c//4 and heads
[4g, 4g+4) where g = c % 4. QKV projection is column-sliced per core,
attention is fully local per head, and the output projection is
row-parallel: each core writes a full-shape [T, D] partial (bf16) and the
host sums the 4 partials per batch.

On-device layout: q,k live transposed as [head_dim, T] so score tiles are
S^T[k, q], softmax normalization is per-column, and the PV matmul consumes
exp(S^T) directly with v in natural [T, head_dim] layout. All matmuls in
bf16 (1 cycle/row on the PE, 1024-wide moving operands); v carries an
extra ones-column so the PV matmul also produces softmax denominators.
Scores accumulate in [128, 1024] PSUM tiles so exp runs as few, wide
ScalarE activations. DMA dispatch is spread across the Sync (x), ScalarE
(weights/consts) and GpSimd (outputs) queues.
"""
import sys

sys.path.insert(0, "/opt/trn_rl_repo")

import numpy as np

B, T, D, H, HD = 2, 2048, 1024, 16, 64
NCORES = 8
GH = 4  # heads per core
DT = 128  # contraction chunk
NDT = D // DT  # 8
KT = 128  # k-tile (score partition dim)
NKT = T // KT  # 16
QC = 1024  # q-chunk width (score free dim / psum tile width)
NQC = T // QC  # 2

_CACHE = {}


def _build():
    import concourse.bass as bass  # noqa: F401
    from concourse import bacc
    import concourse.mybir as mybir
    from concourse.tile import TileContext

    F32 = mybir.dt.float32
    BF16 = mybir.dt.bfloat16
    AF = mybir.ActivationFunctionType

    nc = bacc.Bacc("TRN2", target_bir_lowering=False)

    XT = nc.dram_tensor("xt", [D, T], BF16, kind="ExternalInput")
    # cols: q01 [0:128] | k01 [128:256] | q23 [256:384] | k23 [384:512] | v [512:768]
    WQKV = nc.dram_tensor("wqkv", [D, 768], BF16, kind="ExternalInput")
    WOUT = nc.dram_tensor("wout", [256, D], BF16, kind="ExternalInput")
    COS = nc.dram_tensor("cos2", [128, T], BF16, kind="ExternalInput")
    SIN = nc.dram_tensor("sin2", [128, T], BF16, kind="ExternalInput")
    P2T = nc.dram_tensor("p2t", [128, 128], BF16, kind="ExternalInput")
    TRIMASK = nc.dram_tensor("trimask", [128, 128], BF16, kind="ExternalInput")
    ONESBC = nc.dram_tensor("onesbc", [1, 64], BF16, kind="ExternalInput")
    OUTP = nc.dram_tensor("outp", [T, D], BF16, kind="ExternalOutput")

    with TileContext(nc) as tc:
        with (
            tc.tile_pool(name="const", bufs=1) as cst,
            tc.tile_pool(name="xt", bufs=1) as xtp,
            tc.tile_pool(name="qk", bufs=1) as qkp,
            tc.tile_pool(name="rt", bufs=2) as rtp,
            tc.tile_pool(name="v", bufs=1) as vp,
            tc.tile_pool(name="pt", bufs=8) as ptp,
            tc.tile_pool(name="sm", bufs=2) as smp,
            tc.tile_pool(name="ot", bufs=1) as otp,
            tc.tile_pool(name="os", bufs=3) as osp,
            tc.tile_pool(name="psS", bufs=2, space="PSUM") as psS,
            tc.tile_pool(name="psPV", bufs=2, space="PSUM") as psPV,
            tc.tile_pool(name="psA", bufs=2, space="PSUM") as psA,
        ):
            # ---- const tiles; DMAs ride the ScalarE (ACT) queue ----
            p2t = cst.tile([128, 128], BF16, tag="p2t")
            nc.scalar.dma_start(p2t[:], P2T[:])  # first: feeds the PE warm-up
            wqkv = []
            for d in range(NDT):
                t_ = cst.tile([DT, 768], BF16, tag=f"wqkv{d}", name=f"wqkv{d}")
                nc.scalar.dma_start(t_[:, 0:256], WQKV[d * DT : (d + 1) * DT, 0:256])
                wqkv.append(t_)
            cos = cst.tile([128, T], BF16, tag="cos")
            sin = cst.tile([128, T], BF16, tag="sin")
            trimask = cst.tile([128, 128], BF16, tag="trimask")
            onesbc = cst.tile([1, 64], BF16, tag="onesbc")
            wout = []
            for c2 in range(2):
                nc.scalar.dma_start(cos[:, c2 * QC : (c2 + 1) * QC],
                                    COS[:, c2 * QC : (c2 + 1) * QC])
                nc.scalar.dma_start(sin[:, c2 * QC : (c2 + 1) * QC],
                                    SIN[:, c2 * QC : (c2 + 1) * QC])
            nc.scalar.dma_start(trimask[:], TRIMASK[:])
            nc.scalar.dma_start(onesbc[:], ONESBC[:])
            for d in range(NDT):
                nc.scalar.dma_start(wqkv[d][:, 256:768],
                                    WQKV[d * DT : (d + 1) * DT, 256:768])
            for g in range(2):
                t_ = cst.tile([128, D], BF16, tag=f"wout{g}", name=f"wout{g}")
                nc.scalar.dma_start(t_[:], WOUT[g * 128 : (g + 1) * 128, :])
                wout.append(t_)

            # ---- x tiles, d-major so QKV can stream; dispatch split across
            # the Sync and GpSimd queues (dispatch rate is the feed limit) ----
            xt = []
            for d in range(NDT):
                t_ = xtp.tile([DT, T], BF16, tag=f"xt{d}", name=f"xt{d}")
                xt.append(t_)
            # quarter-major: the whole prefix (q01/k01 seg 0 + v tiles 0-3)
            # consumes only token columns [0:512], so land those first
            for q4 in range(4):
                for d in range(NDT):
                    eng = nc.sync if d % 2 == 0 else nc.gpsimd
                    eng.dma_start(
                        xt[d][:, q4 * 512 : (q4 + 1) * 512],
                        XT[d * DT : (d + 1) * DT, q4 * 512 : (q4 + 1) * 512],
                    )

            # ---- HAM warm-up: dummy matmuls keep the PE busy while x DMAs
            # land, so the clock gate is at 8/8 when real work starts ----
            with nc.named_scope("warmup"):
                wps = psS.tile([128, QC], F32, tag="big", name="warm")
                for _ in range(56):
                    nc.tensor.matmul(
                        wps[:, 0:128], p2t[:], p2t[:], start=True, stop=True
                    )

            # ---- QKV projection: one pair = 2 heads' worth of q or k ----
            qk = {}

            def project_pair(name, col0):
                dst = qkp.tile([128, T], BF16, tag=name, name=name)
                pss = [
                    psS.tile([128, QC], F32, tag="big", name=f"{name}c{c}")
                    for c in range(NQC)
                ]
                for d in range(NDT):
                    for c in range(NQC):
                        for s in range(2):  # matmul out must fit one PSUM bank
                            nc.tensor.matmul(
                                pss[c][:, s * 512 : (s + 1) * 512],
                                wqkv[d][:, col0 : col0 + 128],
                                xt[d][:, c * QC + s * 512 : c * QC + (s + 1) * 512],
                                start=(d == 0),
                                stop=(d == NDT - 1),
                            )
                for c in range(NQC):
                    # split each PSUM->SBUF copy across DVE + ACT (idle here)
                    nc.vector.tensor_copy(
                        dst[:, c * QC : c * QC + 512], pss[c][:, 0:512]
                    )
                    nc.scalar.copy(
                        dst[:, c * QC + 512 : (c + 1) * QC], pss[c][:, 512:QC]
                    )
                qk[name] = dst

            def rope_pair(name):
                raw = qk[name]
                for c in range(NQC):
                    sl = slice(c * QC, (c + 1) * QC)
                    psr = psS.tile([128, QC], F32, tag="big", name=f"r{name}{c}")
                    for s in range(2):
                        nc.tensor.matmul(
                            psr[:, s * 512 : (s + 1) * 512],
                            p2t[:],
                            raw[:, c * QC + s * 512 : c * QC + (s + 1) * 512],
                            start=True,
                            stop=True,
                        )
                    t1 = rtp.tile([128, QC], BF16, tag="t1")
                    nc.vector.tensor_mul(t1[:], psr[:], sin[:, sl])
                    t2 = rtp.tile([128, QC], BF16, tag="t2")
                    nc.vector.tensor_mul(t2[:], raw[:, sl], cos[:, sl])
                    nc.vector.tensor_add(raw[:, sl], t1[:], t2[:])

            # q01/k01: only segment 0 built up front (all block j=0 needs);
            # segments 1-3 stream in as deadline-ordered attention fillers

            # ---- v in natural [tok, vdim] layout, plus ones columns.
            # Tiles 0-3 are built up front; the rest stream in as filler
            # chains inside the attention loop (aux PSUM tag, 1 bank) ----
            vt = [None] * NKT

            def make_vproj(ti):
                def emit():
                    ps = psA.tile([128, 512], F32, tag="aux", name=f"v{ti}")
                    for d in range(NDT):
                        nc.tensor.matmul(
                            ps[:, 0:256],
                            xt[d][:, ti * KT : (ti + 1) * KT],
                            wqkv[d][:, 512:768],
                            start=(d == 0),
                            stop=(d == NDT - 1),
                        )
                    v_ = vp.tile([128, 260], BF16, tag=f"v{ti}", name=f"v{ti}")
                    nc.vector.memset(v_[:], 1.0)
                    for h in range(GH):
                        nc.vector.tensor_copy(
                            v_[:, 65 * h : 65 * h + 64], ps[:, 64 * h : 64 * h + 64]
                        )
                    vt[ti] = v_
                return emit

            # 512-col slice of a q/k projection as one filler chain
            def make_proj_seg(name, col0, s):
                def emit():
                    if qk.get(name) is None:
                        qk[name] = qkp.tile([128, T], BF16, tag=name, name=name)
                    dst = qk[name]
                    ps = psA.tile([128, 512], F32, tag="aux", name=f"{name}s{s}")
                    for d in range(NDT):
                        nc.tensor.matmul(
                            ps[:],
                            wqkv[d][:, col0 : col0 + 128],
                            xt[d][:, s * 512 : (s + 1) * 512],
                            start=(d == 0),
                            stop=(d == NDT - 1),
                        )
                    nc.vector.tensor_copy(dst[:, s * 512 : (s + 1) * 512], ps[:])
                return emit

            def make_rope_seg(name, s):
                def emit():
                    raw = qk[name]
                    sl = slice(s * 512, (s + 1) * 512)
                    psr = psA.tile([128, 512], F32, tag="aux", name=f"r{name}{s}")
                    nc.tensor.matmul(psr[:], p2t[:], raw[:, sl], start=True, stop=True)
                    t1 = rtp.tile([128, 512], BF16, tag="t1s")
                    nc.vector.tensor_mul(t1[:], psr[:], sin[:, sl])
                    t2 = rtp.tile([128, 512], BF16, tag="t2s")
                    nc.vector.tensor_mul(t2[:], raw[:, sl], cos[:, sl])
                    nc.vector.tensor_add(raw[:, sl], t1[:], t2[:])
                return emit

            with nc.named_scope("prefix0"):
                make_proj_seg("q01", 0, 0)()
                make_proj_seg("k01", 128, 0)()
                make_rope_seg("q01", 0)()
                make_rope_seg("k01", 0)()
                for ti in range(4):
                    make_vproj(ti)()

            # ---- attention per head; ot = normalized per-head outputs ----
            ot = [otp.tile([128, T], BF16, tag=f"ot{g}", name=f"ot{g}") for g in range(2)]

            # normalize: row 64 of pso holds sum(exp); fold 1/sum into ot.
            # Deferred so the PE can race ahead into the next block's scores
            # before paying the bcast-matmul dependency on the DVE den copy.
            pending_norm = []

            def flush_norm():
                while pending_norm:
                    h, j, pso = pending_norm.pop(0)
                    pair, hr = h // 2, 64 * (h % 2)
                    den = smp.tile([1, 512], BF16, tag="den")
                    nc.vector.tensor_copy(den[:], pso[64:65, :])
                    psb = psA.tile([128, 512], F32, tag="aux", name=f"bc{h}{j}")
                    nc.tensor.matmul(
                        psb[0:64, :], onesbc[:], den[:], start=True, stop=True
                    )
                    rec = smp.tile([64, 512], F32, tag="rec")
                    nc.vector.reciprocal_approx_fast(rec[:], psb[0:64, :])
                    nc.vector.tensor_mul(
                        ot[pair][hr : hr + 64, j * 512 : (j + 1) * 512],
                        pso[0:64, :],
                        rec[:],
                    )

            def attn_pair(h0, fillers, fillers_by_block=None):
                # 512-wide q-blocks; both heads share one score tile per
                # k-tile ([o:512] head0, [512+o:1024] head1) so each
                # iteration costs ONE exp. PE gaps absorb filler chains.
                pair = h0 // 2
                qT = qk[f"q{'01' if pair == 0 else '23'}"]
                kT = qk[f"k{'01' if pair == 0 else '23'}"]
                hr0, hr1 = 0, 64
                for j in range(4):
                    n_i = 4 * j + 4
                    pso = {
                        h: psPV.tile([65, 512], F32, tag="pv", name=f"pso{h}j{j}")
                        for h in (h0, h0 + 1)
                    }
                    fifo = []  # software pipeline: PV trails scores by 1 iter
                    for i in range(n_i):
                        ob = i * KT - j * 512
                        o = max(0, ob)
                        qsl = slice(j * 512 + o, (j + 1) * 512)
                        ps = psS.tile([128, QC], F32, tag="big", name=f"s{h0}_{j}_{i}")
                        nc.tensor.matmul(
                            ps[:, o:512],
                            kT[hr0 : hr0 + 64, i * KT : (i + 1) * KT],
                            qT[hr0 : hr0 + 64, qsl],
                            start=True,
                            stop=True,
                        )
                        # head1 packs at [512 : 1024-o] so the merged exp
                        # below is gap-free (no stale-psum columns)
                        nc.tensor.matmul(
                            ps[:, 512 : 1024 - o],
                            kT[hr1 : hr1 + 64, i * KT : (i + 1) * KT],
                            qT[hr1 : hr1 + 64, qsl],
                            start=True,
                            stop=True,
                        )
                        if i == 0:
                            flush_norm()
                        pt = ptp.tile([128, QC], BF16, tag="pt", name=f"pt{h0}_{j}_{i}")
                        nc.scalar.activation(
                            pt[:, o : 1024 - o], ps[:, o : 1024 - o], AF.Exp, scale=0.125
                        )
                        if ob >= 0:
                            nc.vector.tensor_mul(
                                pt[:, o : o + 128], pt[:, o : o + 128], trimask[:]
                            )
                            nc.vector.tensor_mul(
                                pt[:, 512:640], pt[:, 512:640], trimask[:]
                            )
                        if fillers:
                            fillers.pop(0)()
                        elif fillers_by_block:
                            # constrained fillers: only chains whose source
                            # block is already normalized (min_block <= j)
                            for idx, (mb, emit) in enumerate(fillers_by_block):
                                if mb <= j:
                                    fillers_by_block.pop(idx)
                                    emit()
                                    break
                        fifo.append((i, pt, o))
                        if len(fifo) > 1:
                            emit_pv(h0, j, pso, n_i, fifo.pop(0))
                    while fifo:
                        emit_pv(h0, j, pso, n_i, fifo.pop(0))
                    for h in (h0, h0 + 1):
                        pending_norm.append((h, j, pso[h]))

            def emit_pv(h0, j, pso, n_i, item):
                i, pt, o = item
                for h, m0, m1 in ((h0, o, 512), (h0 + 1, 512, 1024 - o)):
                    nc.tensor.matmul(
                        pso[h][:, o:512],
                        vt[i][:, 65 * h : 65 * h + 65],
                        pt[:, m0:m1],
                        start=(i == 0),
                        stop=(i == n_i - 1),
                    )

            # full output-projection for one token tile as one filler chain.
            # ACT is the attention bottleneck, so mid-attention chains keep
            # their PSUM->SBUF copy entirely on DVE; tail chains split.
            def make_oproj(ti, use_act=False):
                def emit():
                    ps = psS.tile([128, QC], F32, tag="big", name=f"o{ti}")
                    for g in range(2):
                        for s in range(2):
                            nc.tensor.matmul(
                                ps[:, s * 512 : (s + 1) * 512],
                                ot[g][:, ti * KT : (ti + 1) * KT],
                                wout[g][:, s * 512 : (s + 1) * 512],
                                start=(g == 0),
                                stop=(g == 1),
                            )
                    osb = osp.tile([128, D], BF16, tag="ost")
                    nc.vector.tensor_copy(osb[:, 0:512], ps[:, 0:512])
                    if use_act:
                        nc.scalar.copy(osb[:, 512:1024], ps[:, 512:1024])
                    else:
                        nc.vector.tensor_copy(osb[:, 512:1024], ps[:, 512:1024])
                    for half in range(2):
                        nc.gpsimd.dma_start(
                            OUTP[ti * KT : (ti + 1) * KT, half * 512 : (half + 1) * 512],
                            osb[:, half * 512 : (half + 1) * 512],
                        )
                return emit

            # deadline-ordered: block j of pair01 needs q01/k01 segment j
            # (roped) by global iteration 4+8j(j-1)/... — verified: seg s
            # chains pop at iters 8(s-1)..8(s-1)+3, consumed from iter
            # 4+Σ; v tiles pop ≥2 iters before their first PV
            fill01 = []
            for s in (1, 2, 3):
                fill01 += [
                    make_proj_seg("q01", 0, s),
                    make_proj_seg("k01", 128, s),
                    make_rope_seg("q01", s),
                    make_rope_seg("k01", s),
                    make_vproj(4 * s),
                    make_vproj(4 * s + 1),
                    make_vproj(4 * s + 2),
                    make_vproj(4 * s + 3),
                ]
            fill01 += (
                [make_proj_seg("q23", 256, s) for s in range(4)]
                + [make_proj_seg("k23", 384, s) for s in range(4)]
                + [make_rope_seg("q23", s) for s in range(4)]
                + [make_rope_seg("k23", s) for s in range(4)]
            )
            with nc.named_scope("attn01"):
                attn_pair(0, fill01)
            for f in fill01:  # anything the 40 iterations didn't absorb
                f()
            # oproj(ti) may only run once pair23's block ti//4 is normalized
            # (one block later); ot's pair01 rows are final by then
            fill23 = [
                (ti // 4 + 1, make_oproj(ti, use_act=(ti >= 12))) for ti in range(NKT)
            ]
            with nc.named_scope("attn23"):
                attn_pair(2, [], fill23)

            # ---- tail: remaining oproj tiles (last pair23 block) ----
            with nc.named_scope("oproj"):
                flush_norm()
                for _, f in fill23:
                    f()

    nc.compile()
    return nc


def _host_consts(bf16):
    pos = np.arange(T, dtype=np.float64)
    theta = 1.0 / (10000.0 ** (np.arange(0, HD, 2, dtype=np.float64) / HD))
    ang = pos[:, None] * theta[None, :]  # [T, 32]
    cos = np.tile(np.cos(ang), (1, 2)).T  # [64, T]
    sin = np.tile(np.sin(ang), (1, 2)).T
    cos2 = np.vstack([cos, cos]).astype(bf16)  # [128, T] two heads stacked
    sin2 = np.vstack([sin, sin]).astype(bf16)
    # rotate-half as a matmul: rot = P @ q for q in [64, t] column layout
    P = np.zeros((HD, HD), dtype=np.float32)
    for i_ in range(32):
        P[i_, i_ + 32] = -1.0
        P[i_ + 32, i_] = 1.0
    P2 = np.zeros((128, 128), dtype=np.float32)
    P2[0:64, 0:64] = P
    P2[64:128, 64:128] = P
    p2t = np.ascontiguousarray(P2.T).astype(bf16)
    f, p = np.meshgrid(np.arange(128), np.arange(128))
    trimask = (p <= f).astype(bf16)  # [p, f] valid iff p <= f
    onesbc = np.ones((1, 64), dtype=np.float32).astype(bf16)
    return cos2, sin2, p2t, trimask, onesbc


def kernel(x, w_qkv, w_out, b_out):
    import ml_dtypes
    from concourse.bass_utils import run_bass_kernel_spmd

    bf16 = ml_dtypes.bfloat16

    if "nc" not in _CACHE:
        _CACHE["nc"] = _build()
    nc = _CACHE["nc"]

    x = np.asarray(x, dtype=np.float32)
    w_qkv = np.asarray(w_qkv, dtype=np.float32)
    w_out = np.asarray(w_out, dtype=np.float32)
    b_out = np.asarray(b_out, dtype=np.float32)

    cos2, sin2, p2t, trimask, onesbc = _host_consts(bf16)

    wq = w_qkv[:, 0:D]
    wk = w_qkv[:, D : 2 * D]
    wv = w_qkv[:, 2 * D : 3 * D]
    xt_b = [np.ascontiguousarray(x[b].T).astype(bf16) for b in range(B)]

    in_maps = []
    for c in range(NCORES):
        b, g = c // 4, c % 4
        h0 = GH * g  # first head of this core's group
        cs = slice(h0 * HD, h0 * HD + 128)  # heads h0, h0+1
        cs2 = slice(h0 * HD + 128, h0 * HD + 256)  # heads h0+2, h0+3
        vs = slice(h0 * HD, h0 * HD + 256)
        wqkv_c = np.ascontiguousarray(
            np.concatenate([wq[:, cs], wk[:, cs], wq[:, cs2], wk[:, cs2], wv[:, vs]], axis=1)
        ).astype(bf16)  # [D, 768]
        wout_c = np.ascontiguousarray(w_out[vs, :]).astype(bf16)  # [256, D]
        in_maps.append(
            {
                "xt": xt_b[b],
                "wqkv": wqkv_c,
                "wout": wout_c,
                "cos2": cos2,
                "sin2": sin2,
                "p2t": p2t,
                "trimask": trimask,
                "onesbc": onesbc,
            }
        )

    global _last_in_maps
    _last_in_maps = in_maps
    res = run_bass_kernel_spmd(nc, in_maps, list(range(NCORES)))
    out = np.zeros((B, T, D), dtype=np.float64)
    for c in range(NCORES):
        out[c // 4] += np.asarray(res.results[c]["outp"]).astype(np.float64)
    out += b_out.astype(np.float64)
    return out.astype(np.float32)
